# revision 18
# baseline (speedup 1.0000x reference)
"""Trainium2 Bass kernel: KDeep-style 3D CNN with atom->voxel splat.

Strategy (8 NeuronCores, pure data parallel, 2 batches/core):
  splat:  s = d^2 (via one K=5 matmul vs the voxel grid),
          x = (vr/d)^12 = Exp(-6*Ln(s) + 12*ln(vr)),  e = Exp(-x),  c = 1-e,
          lat[f,v] = sum_n h[n,f] * c[n,v]  (matmul over atoms)
  cnn:    conv via kernel-offset accumulation on the PE; fire concat([e,e])
          folded into halved squeeze weights; pools on DVE; final dot on PE.
"""
import os
import sys

import numpy as np

for _p in ("/opt/trn_rl_repo",):
    if _p not in sys.path:
        sys.path.insert(0, _p)

import ml_dtypes  # noqa: E402

BF = ml_dtypes.bfloat16

B, N1, N2, F, L = 16, 300, 40, 54, 24
NA = N1 + N2          # 340 atoms per batch (pos1 ++ pos2)
V = L ** 3            # 13824 voxels
NCORES, BPC = 8, 2    # batches per core
HALF = 12.0           # LATTICE_DIM / 2
NSTACK = 3            # grid stacks at partition bases 0/32/64
VSTACK = V // NSTACK  # 4608
VCH = 1024            # voxel chunk for elementwise splat work
ACH = (128, 128, 84)  # atom chunks
AOFF = (0, 128, 256)

# fire name -> (cin_raw, squeeze, expand)
FIRE = {'2': (96, 16, 64), '3': (128, 16, 64), '4': (128, 32, 128),
        '5': (256, 32, 128), '6': (256, 48, 192), '7': (384, 48, 192),
        '8': (384, 64, 256)}
# effective (folded) squeeze input channels
CIN_EFF = {'2': 96, '3': 64, '4': 64, '5': 128, '6': 128, '7': 192, '8': 192}

DEBUG_LAT = bool(int(os.environ.get("KD_DEBUG_LAT", "0")))

_CACHE = {}


def _f32(a):
    return np.ascontiguousarray(np.asarray(a, dtype=np.float32))


def _bf(a):
    return np.ascontiguousarray(np.asarray(a, dtype=np.float32).astype(BF))


def _host_consts(params):
    """Input-independent constant prep: grid table, folded/transposed weights."""
    c = {}
    # one-hot distance-gather table: row a*24+t is 1 where coordinate a of
    # voxel v equals t. d2[n,v] = sum_rows D2[n,row] * onehot[row,v] is exact.
    idx = np.arange(L, dtype=np.float32)
    gx, gy, gz = np.meshgrid(idx, idx, idx, indexing='ij')
    g = np.stack([gx.ravel(), gy.ravel(), gz.ravel()], 0)         # [3, V]
    oh = np.zeros((72, V), np.float32)
    for a in range(3):
        for t in range(L):
            oh[a * L + t] = (g[a] == t)
    c["onehot"] = _bf(oh)
    c["iota72"] = _f32((np.arange(72) % L).reshape(72, 1))

    def lhsT(w):  # [O, I, 3,3,3] -> [I, 27, O]
        w = np.asarray(w, np.float32)
        o, i = w.shape[:2]
        return np.transpose(w.reshape(o, i, 27), (1, 2, 0))

    w1 = np.asarray(params['conv1_w'], np.float32)                # [96,54,2,2,2]
    c["w_c1"] = _bf(np.transpose(w1.reshape(96, 54, 8), (1, 2, 0)))  # [54,8,96]
    c["b_c1"] = _f32(params['conv1_b']).reshape(-1, 1)

    for n, (cin_raw, sq, ex) in FIRE.items():
        wsq = np.asarray(params[f'sq{n}_w'], np.float32)
        if n != '2':
            e = CIN_EFF[n]
            wsq = wsq[:, :e] + wsq[:, e:]
        wt = lhsT(wsq)                                            # [cin_eff,27,sq]
        if wt.shape[0] > 128:
            c[f"w_sq{n}_0"] = _bf(wt[:128])
            c[f"w_sq{n}_1"] = _bf(wt[128:])
        else:
            c[f"w_sq{n}_0"] = _bf(wt)
        c[f"b_sq{n}"] = _f32(params[f'sq{n}_b']).reshape(-1, 1)
        c[f"w_ex{n}_0"] = _bf(lhsT(np.asarray(params[f'ex{n}_w'], np.float32)))
        be = _f32(params[f'ex{n}_b']).reshape(-1, 1)
        for i in range(0, be.shape[0], 128):
            c[f"b_ex{n}_{i // 128}"] = be[i:i + 128]

    lw = np.asarray(params['lin_w'], np.float32).reshape(512, 8)
    lw2 = (lw[:256] + lw[256:]) / 27.0                            # [256, 8]
    c["lw_0"] = _f32(lw2[:128])
    c["lw_1"] = _f32(lw2[128:])
    c["lin_b"] = _f32(params['lin_b']).reshape(1, 1)
    c["ones128"] = _f32(np.ones((128, 1)))
    return c


def _build_nc():
    import concourse.bacc as bacc
    import concourse.tile as tile
    from concourse import mybir

    DT = mybir.dt
    AF = mybir.ActivationFunctionType
    ALU = mybir.AluOpType

    nc = bacc.Bacc("TRN2", target_bir_lowering=False, debug=False,
                   num_devices=NCORES)

    # ------------------- DRAM I/O -------------------
    p72_d = nc.dram_tensor("p72", [BPC, 72, NA], DT.float32, kind="ExternalInput")
    vr_d = nc.dram_tensor("vr", [BPC, NA, 1], DT.float32, kind="ExternalInput")
    hbf_d = nc.dram_tensor("hbf", [BPC, NA, F], DT.bfloat16, kind="ExternalInput")
    onehot_d = nc.dram_tensor("onehot", [72, V], DT.bfloat16, kind="ExternalInput")
    iota72_d = nc.dram_tensor("iota72", [72, 1], DT.float32, kind="ExternalInput")
    w_c1_d = nc.dram_tensor("w_c1", [54, 8, 96], DT.bfloat16, kind="ExternalInput")
    b_c1_d = nc.dram_tensor("b_c1", [96, 1], DT.float32, kind="ExternalInput")
    wd, bd = {}, {}
    for n, (cin_raw, sq, ex) in FIRE.items():
        ce = CIN_EFF[n]
        if ce > 128:
            wd[f"sq{n}"] = [nc.dram_tensor(f"w_sq{n}_0", [128, 27, sq], DT.bfloat16, kind="ExternalInput"),
                            nc.dram_tensor(f"w_sq{n}_1", [ce - 128, 27, sq], DT.bfloat16, kind="ExternalInput")]
        else:
            wd[f"sq{n}"] = [nc.dram_tensor(f"w_sq{n}_0", [ce, 27, sq], DT.bfloat16, kind="ExternalInput")]
        wd[f"ex{n}"] = [nc.dram_tensor(f"w_ex{n}_0", [sq, 27, ex], DT.bfloat16, kind="ExternalInput")]
        bd[f"sq{n}"] = [nc.dram_tensor(f"b_sq{n}", [sq, 1], DT.float32, kind="ExternalInput")]
        bd[f"ex{n}"] = [nc.dram_tensor(f"b_ex{n}_{i // 128}",
                                       [min(128, ex - i), 1], DT.float32,
                                       kind="ExternalInput")
                        for i in range(0, ex, 128)]
    lw0_d = nc.dram_tensor("lw_0", [128, 8], DT.float32, kind="ExternalInput")
    lw1_d = nc.dram_tensor("lw_1", [128, 8], DT.float32, kind="ExternalInput")
    linb_d = nc.dram_tensor("lin_b", [1, 1], DT.float32, kind="ExternalInput")
    ones128_d = nc.dram_tensor("ones128", [128, 1], DT.float32, kind="ExternalInput")
    out_d = nc.dram_tensor("out", [BPC, 1], DT.float32, kind="ExternalOutput")
    if DEBUG_LAT:
        latdbg_d = nc.dram_tensor("lat_dbg", [F, BPC * V], DT.bfloat16,
                                  kind="ExternalOutput")

    with tile.TileContext(nc) as tc:
        with (
            tc.tile_pool(name="const", bufs=1) as cpool,
            tc.tile_pool(name="prep", bufs=1) as prep,
            tc.tile_pool(name="ew", bufs=2) as ew,
            tc.tile_pool(name="cbf", bufs=2) as cbf,
            tc.tile_pool(name="lat", bufs=1) as latp,
            tc.tile_pool(name="cnn", bufs=3) as cnnp,
            tc.tile_pool(name="e4p", bufs=1) as e4p,
            tc.tile_pool(name="mpp", bufs=2) as mpp,
            tc.tile_pool(name="p7p", bufs=6) as p7p,
            tc.tile_pool(name="e8p", bufs=2) as e8p,
            tc.tile_pool(name="app", bufs=2) as app,
            tc.tile_pool(name="small", bufs=4) as smallp,
            tc.tile_pool(name="ps_s", bufs=2, space="PSUM") as ps_s,
            tc.tile_pool(name="ps_lat", bufs=1, space="PSUM") as ps_lat,
            tc.tile_pool(name="ps_cnn", bufs=2, space="PSUM") as ps_cnn,
        ):
            # ---------------- constants in ----------------
            oh_t = cpool.tile([72, V], DT.bfloat16, tag="onehot")
            nc.sync.dma_start(oh_t[:], onehot_d[:])
            iota72_t = cpool.tile([72, 1], DT.float32, tag="iota72")
            nc.sync.dma_start(iota72_t[:], iota72_d[:])
            wc1_t = cpool.tile([54, 8, 96], DT.bfloat16, tag="wc1")
            nc.sync.dma_start(wc1_t[:], w_c1_d[:])
            bc1_t = cpool.tile([96, 1], DT.float32, tag="bc1")
            nc.sync.dma_start(bc1_t[:], b_c1_d[:])
            wt, bt = {}, {}
            for key, dlist in wd.items():
                wt[key] = []
                for i, d in enumerate(dlist):
                    t = cpool.tile(list(d.shape), DT.bfloat16, tag=f"w{key}{i}")
                    nc.sync.dma_start(t[:], d[:])
                    wt[key].append(t)
            for key, dlist in bd.items():
                bt[key] = []
                for i, d in enumerate(dlist):
                    t = cpool.tile(list(d.shape), DT.float32, tag=f"b{key}{i}")
                    nc.sync.dma_start(t[:], d[:])
                    bt[key].append(t)
            lw_t0 = cpool.tile([128, 8], DT.float32, tag="lw0")
            lw_t1 = cpool.tile([128, 8], DT.float32, tag="lw1")
            lw_t = [lw_t0, lw_t1]
            nc.sync.dma_start(lw_t[0][:], lw0_d[:])
            nc.sync.dma_start(lw_t[1][:], lw1_d[:])
            linb_t = cpool.tile([1, 1], DT.float32, tag="linb")
            nc.sync.dma_start(linb_t[:], linb_d[:])
            ones128_t = cpool.tile([128, 1], DT.float32, tag="ones128")
            nc.sync.dma_start(ones128_t[:], ones128_d[:])

            # ---------------- per-batch atom prep ----------------
            # D2 rows a*24+t = (p_a + shift_a - t)^2, split hi/mid/lo into
            # three bf16 tables so the one-hot matmul reconstructs exact d^2.
            qhi_t = prep.tile([72, BPC * NA], DT.bfloat16, tag="qhi")
            qr1_t = prep.tile([72, BPC * NA], DT.bfloat16, tag="qr1")
            qr2_t = prep.tile([72, BPC * NA], DT.bfloat16, tag="qr2")
            bias12 = {}   # (b, ac) -> [aclen,1] tile with 12*ln(vr)
            h_t = {}      # (b, ac) -> [aclen, F] bf16
            for b in range(BPC):
                p72 = prep.tile([72, NA], DT.float32, tag="p72")
                nc.sync.dma_start(p72[:], p72_d[b])
                bmax = smallp.tile([72, 1], DT.float32, tag="bmax")
                bmin = smallp.tile([72, 1], DT.float32, tag="bmin")
                nc.vector.tensor_reduce(bmax[:], p72[:, :N1], axis=mybir.AxisListType.X,
                                        op=ALU.max)
                nc.vector.tensor_reduce(bmin[:], p72[:, :N1], axis=mybir.AxisListType.X,
                                        op=ALU.min)
                sh = smallp.tile([72, 1], DT.float32, tag="sh")
                nc.vector.tensor_add(sh[:], bmax[:], bmin[:])
                # shift = -(bmin+bmax)/2 + HALF; bias row = shift_a - t
                nc.vector.tensor_scalar(sh[:], sh[:], -0.5, HALF, ALU.mult, ALU.add)
                nc.vector.tensor_sub(sh[:], sh[:], iota72_t[:])
                d2f = prep.tile([72, NA], DT.float32, tag="d2f")
                nc.scalar.activation(d2f[:], p72[:], AF.Square, bias=sh[:])
                bsl = slice(b * NA, (b + 1) * NA)
                nc.vector.tensor_copy(qhi_t[:, bsl], d2f[:])
                r1 = prep.tile([72, NA], DT.float32, tag="r1")
                nc.vector.tensor_sub(r1[:], d2f[:], qhi_t[:, bsl])
                nc.vector.tensor_copy(qr1_t[:, bsl], r1[:])
                nc.vector.tensor_sub(r1[:], r1[:], qr1_t[:, bsl])
                nc.vector.tensor_copy(qr2_t[:, bsl], r1[:])

                for ac in range(3):
                    al, ao = ACH[ac], AOFF[ac]
                    vrt = smallp.tile([al, 1], DT.float32, tag="vrt")
                    nc.sync.dma_start(vrt[:], vr_d[b, ao:ao + al])
                    vr2 = smallp.tile([al, 1], DT.float32, tag="vr2")
                    nc.vector.tensor_mul(vr2[:], vrt[:], vrt[:])
                    lnv = smallp.tile([al, 1], DT.float32, tag="lnv")
                    nc.scalar.activation(lnv[:], vr2[:], AF.Ln)
                    b12 = prep.tile([al, 1], DT.float32, tag=f"b12_{b}_{ac}")
                    nc.vector.tensor_scalar_mul(b12[:], lnv[:], 6.0)
                    bias12[(b, ac)] = b12
                    ht = prep.tile([al, F], DT.bfloat16, tag=f"h_{b}_{ac}")
                    nc.sync.dma_start(ht[:], hbf_d[b, ao:ao + al, :])
                    h_t[(b, ac)] = ht

            # ---------------- splat ----------------
            lat_t = latp.tile([F, BPC, L, L, L], DT.bfloat16, tag="lat")
            lat_fl = lat_t[:].rearrange("f b i j k -> f (b i j k)")
            for b in range(BPC):
                voff = 0
                while voff < V:
                    cl = min(VCH, V - voff)
                    latps = ps_lat.tile([F, VCH], DT.float32, tag="latps")
                    for ac in range(3):
                        al, ao = ACH[ac], AOFF[ac]
                        asl = slice(b * NA + ao, b * NA + ao + al)
                        sps = ps_s.tile([128, VCH], DT.float32, tag="sps")
                        for j in range(0, cl, 512):
                            je = min(j + 512, cl)
                            for si, qt in enumerate((qhi_t, qr1_t, qr2_t)):
                                nc.tensor.matmul(
                                    sps[:al, j:je], qt[:, asl],
                                    oh_t[:, voff + j:voff + je],
                                    start=(si == 0), stop=(si == 2))
                        y = ew.tile([128, VCH], DT.float32, tag="ew")
                        nc.scalar.activation(y[:al, :cl], sps[:al, :cl], AF.Ln)
                        nc.scalar.activation(y[:al, :cl], y[:al, :cl], AF.Exp,
                                             bias=bias12[(b, ac)][:],
                                             scale=-6.0)
                        nc.scalar.activation(y[:al, :cl], y[:al, :cl], AF.Exp,
                                             scale=-1.0)
                        ct = cbf.tile([128, VCH], DT.bfloat16, tag="c")
                        nc.vector.tensor_scalar(ct[:al, :cl], y[:al, :cl],
                                                -1.0, 1.0, ALU.mult, ALU.add)
                        for j in range(0, cl, 512):
                            je = min(j + 512, cl)
                            nc.tensor.matmul(
                                latps[:, j:je], h_t[(b, ac)][:],
                                ct[:al, j:je],
                                start=(ac == 0), stop=(ac == 2))
                    v0 = b * V + voff
                    nc.vector.tensor_copy(lat_fl[:, v0:v0 + cl],
                                          latps[:, :cl])
                    voff += cl
            if DEBUG_LAT:
                nc.sync.dma_start(latdbg_d[:], lat_fl[:])

            # ---------------- CNN ----------------
            def conv_to(psum_pool, in_views, w_tiles, mslice, n_views,
                        first_extra=False):
                """Accumulate sum over kchunks x 27 offsets into psum tiles.
                in_views: per kchunk, fn(t0,t1,t2, nv) -> rhs AP
                n_views:  list of (psum_tile, nv_key) output chunks."""
                for (pt, nv) in n_views:
                    first = True
                    for kc, wtile in enumerate(w_tiles):
                        for t0 in range(3):
                            for t1 in range(3):
                                for t2 in range(3):
                                    t27 = (t0 * 3 + t1) * 3 + t2
                                    rhs = in_views[kc](t0, t1, t2, nv)
                                    last = (kc == len(w_tiles) - 1 and t27 == 26)
                                    nc.tensor.matmul(
                                        pt[:], wtile[:, t27, mslice], rhs,
                                        start=first, stop=last)
                                    first = False

            for b in range(BPC):
                # conv1: lat [54, (i,j,k)] -> pad14 buffer x1 [96, 14^3]
                x1 = cnnp.tile([128, 14, 14, 14], DT.bfloat16, tag="pad14")
                nc.vector.memset(x1[:96], 0.0)
                for gchunk in range(4):
                    a0 = 3 * gchunk
                    pt = ps_cnn.tile([96, 3, 12, 12], DT.float32, tag="cps")
                    first = True
                    for tj in range(2):
                        for tk in range(2):
                            for ti in range(2):
                                t8 = (tj * 2 + tk) * 2 + ti
                                # out (dj,hk,wi): in i=2wi+ti, j=2dj+tj, k=2hk+tk
                                rhs = lat_t[:, b, ti:24:2,
                                            tj + 2 * a0:tj + 2 * a0 + 5:2,
                                            tk:24:2]
                                rhs = rhs.rearrange("f i j k -> f j k i")
                                nc.tensor.matmul(pt[:], wc1_t[:, t8, :], rhs,
                                                 start=first, stop=(t8 == 7))
                                first = False
                    nc.scalar.activation(
                        x1[:96, 1 + a0:4 + a0, 1:13, 1:13], pt[:],
                        AF.Relu, bias=bc1_t[:])

                def pad14_views(t, cin):
                    def f(t0, t1, t2, nv):
                        a0 = nv
                        return t[:cin, a0 + t0:a0 + t0 + 3, t1:t1 + 12,
                                 t2:t2 + 12]
                    return f

                def fire12(name, in_t, cin):
                    """12^3 fire layer pair; returns expand output tile (pad14)."""
                    sq_c = FIRE[name][1]
                    ex_c = FIRE[name][2]
                    s_t = cnnp.tile([128, 14, 14, 14], DT.bfloat16, tag="pad14")
                    nc.vector.memset(s_t[:sq_c], 0.0)
                    for gch in range(4):
                        pt = ps_cnn.tile([sq_c, 3, 12, 12], DT.float32, tag="cps")
                        conv_to(ps_cnn, [pad14_views(in_t, cin)], wt[f"sq{name}"],
                                slice(0, sq_c), [(pt, 3 * gch)])
                        nc.scalar.activation(
                            s_t[:sq_c, 1 + 3 * gch:4 + 3 * gch, 1:13, 1:13],
                            pt[:], AF.Relu, bias=bt[f"sq{name}"][0][:])
                    if name == '4':
                        e_t = e4p.tile([128, 12, 12, 12], DT.bfloat16, tag="e4")
                    else:
                        e_t = cnnp.tile([128, 14, 14, 14], DT.bfloat16, tag="pad14")
                        nc.vector.memset(e_t[:ex_c], 0.0)
                    for gch in range(4):
                        pt = ps_cnn.tile([ex_c, 3, 12, 12], DT.float32, tag="cps")
                        conv_to(ps_cnn, [pad14_views(s_t, sq_c)], wt[f"ex{name}"],
                                slice(0, ex_c), [(pt, 3 * gch)])
                        if name == '4':
                            nc.scalar.activation(
                                e_t[:ex_c, 3 * gch:3 * gch + 3, :, :], pt[:],
                                AF.Relu, bias=bt[f"ex{name}"][0][:])
                        else:
                            nc.scalar.activation(
                                e_t[:ex_c, 1 + 3 * gch:4 + 3 * gch, 1:13, 1:13],
                                pt[:], AF.Relu, bias=bt[f"ex{name}"][0][:])
                    return e_t

                e2 = fire12('2', x1, 96)
                e3 = fire12('3', e2, 64)
                e4 = fire12('4', e3, 64)

                # maxpool(2,3,1): 12 -> 5 per axis, into pad7 [128,7,7,7]
                t1 = mpp.tile([128, 12, 12, 5], DT.bfloat16, tag="mp1")
                nc.vector.tensor_max(t1[:, :, :, 1:4], e4[:, :, :, 2:11:3],
                                     e4[:, :, :, 3:12:3])
                nc.vector.tensor_copy(t1[:, :, :, 0:1], e4[:, :, :, 0:1])
                nc.vector.tensor_copy(t1[:, :, :, 4:5], e4[:, :, :, 11:12])
                t2 = mpp.tile([128, 12, 5, 5], DT.bfloat16, tag="mp2")
                nc.vector.tensor_max(t2[:, :, 1:4, :], t1[:, :, 2:11:3, :],
                                     t1[:, :, 3:12:3, :])
                nc.vector.tensor_copy(t2[:, :, 0:1, :], t1[:, :, 0:1, :])
                nc.vector.tensor_copy(t2[:, :, 4:5, :], t1[:, :, 11:12, :])
                m4 = p7p.tile([128, 7, 7, 7], DT.bfloat16, tag="pad7")
                nc.vector.memset(m4[:], 0.0)
                nc.vector.tensor_max(m4[:, 2:5, 1:6, 1:6], t2[:, 2:11:3, :, :],
                                     t2[:, 3:12:3, :, :])
                nc.vector.tensor_copy(m4[:, 1:2, 1:6, 1:6], t2[:, 0:1, :, :])
                nc.vector.tensor_copy(m4[:, 5:6, 1:6, 1:6], t2[:, 11:12, :, :])

                def pad7_views(t, cin):
                    def f(t0, t1, t2, nv):
                        return t[:cin, t0:t0 + 5, t1:t1 + 5, t2:t2 + 5]
                    return f

                def fire5(name, in_tiles):
                    """5^3 fire; in_tiles: list of (pad7 tile, cin) kchunks.
                    Returns list of output (tile, ch) (plain tiles for f8)."""
                    sq_c = FIRE[name][1]
                    ex_c = FIRE[name][2]
                    s_t = p7p.tile([128, 7, 7, 7], DT.bfloat16, tag="pad7")
                    nc.vector.memset(s_t[:sq_c], 0.0)
                    pt = ps_cnn.tile([sq_c, 5, 5, 5], DT.float32, tag="cps")
                    conv_to(ps_cnn, [pad7_views(t, cc) for (t, cc) in in_tiles],
                            wt[f"sq{name}"], slice(0, sq_c), [(pt, 0)])
                    nc.scalar.activation(s_t[:sq_c, 1:6, 1:6, 1:6], pt[:],
                                         AF.Relu, bias=bt[f"sq{name}"][0][:])
                    outs = []
                    m0 = 0
                    while m0 < ex_c:
                        mc = min(128, ex_c - m0)
                        if name == '8':
                            o_t = e8p.tile([128, 5, 5, 5], DT.bfloat16, tag="e8")
                            ov = o_t[:mc, :, :, :]
                        else:
                            o_t = p7p.tile([128, 7, 7, 7], DT.bfloat16, tag="pad7")
                            nc.vector.memset(o_t[:mc], 0.0)
                            ov = o_t[:mc, 1:6, 1:6, 1:6]
                        pt = ps_cnn.tile([mc, 5, 5, 5], DT.float32, tag="cps")
                        conv_to(ps_cnn, [pad7_views(s_t, sq_c)], wt[f"ex{name}"],
                                slice(m0, m0 + mc), [(pt, 0)])
                        nc.scalar.activation(ov, pt[:], AF.Relu,
                                             bias=bt[f"ex{name}"][m0 // 128][:])
                        outs.append((o_t, mc))
                        m0 += mc
                    return outs

                f5 = fire5('5', [(m4, 128)])
                f6 = fire5('6', f5)
                f7 = fire5('7', f6)
                f8 = fire5('8', f7)

                # avgpool(3,2,0): 5 -> 2 per axis (sum; 1/27 folded into lw)
                fin = ps_cnn.tile([1, 1], DT.float32, tag="cps")
                for hh, (e8, _mc) in enumerate(f8):
                    u1 = app.tile([128, 5, 5, 2], DT.float32, tag="ap1")
                    nc.vector.tensor_add(u1[:], e8[:, :, :, 0:4:2], e8[:, :, :, 1:5:2])
                    nc.vector.tensor_add(u1[:], u1[:], e8[:, :, :, 2:5:2])
                    u2 = app.tile([128, 5, 2, 2], DT.float32, tag="ap2")
                    nc.vector.tensor_add(u2[:], u1[:, :, 0:4:2, :], u1[:, :, 1:5:2, :])
                    nc.vector.tensor_add(u2[:], u2[:], u1[:, :, 2:5:2, :])
                    u3 = app.tile([128, 2, 2, 2], DT.float32, tag="ap3")
                    nc.vector.tensor_add(u3[:], u2[:, 0:4:2, :, :], u2[:, 1:5:2, :, :])
                    nc.vector.tensor_add(u3[:], u3[:], u2[:, 2:5:2, :, :])
                    # dot with folded linear weights
                    m1 = app.tile([128, 8], DT.float32, tag="lm")
                    nc.vector.tensor_mul(m1[:], u3[:].rearrange("p a b c -> p (a b c)"),
                                         lw_t[hh][:])
                    red = app.tile([128, 1], DT.float32, tag="lred")
                    nc.vector.tensor_reduce(red[:], m1[:], axis=mybir.AxisListType.X,
                                            op=ALU.add)
                    nc.tensor.matmul(fin[:], red[:], ones128_t[:],
                                     start=(hh == 0), stop=(hh == 1))
                ob = smallp.tile([1, 1], DT.float32, tag="ob")
                nc.scalar.activation(ob[:], fin[:], AF.Identity, bias=linb_t[:])
                nc.sync.dma_start(out_d[b:b + 1, :], ob[:])

    nc.compile()
    return nc


def _get_nc():
    if "nc" not in _CACHE:
        _CACHE["nc"] = _build_nc()
    return _CACHE["nc"]


def kernel(pos1, pos2, vr1, vr2, h1, h2, params):
    from concourse.bass_utils import run_bass_kernel_spmd

    pos1, pos2 = _f32(pos1), _f32(pos2)
    vr1, vr2 = _f32(vr1), _f32(vr2)
    h1, h2 = _f32(h1), _f32(h2)

    consts = _host_consts(params)
    nc = _get_nc()

    posT = np.concatenate([pos1, pos2], axis=1).transpose(0, 2, 1)  # [B,3,NA]
    p72 = np.repeat(posT, L, axis=1)                                # [B,72,NA]
    vr = np.concatenate([vr1, vr2], axis=1)[..., None]              # [B,NA,1]
    hbf = np.concatenate([h1, h2], axis=1).astype(BF)               # [B,NA,F]

    in_maps = []
    for c in range(NCORES):
        b0 = c * BPC
        m = dict(consts)
        m["p72"] = _f32(p72[b0:b0 + BPC])
        m["vr"] = _f32(vr[b0:b0 + BPC])
        m["hbf"] = np.ascontiguousarray(hbf[b0:b0 + BPC])
        in_maps.append(m)

    res = run_bass_kernel_spmd(nc, in_maps, core_ids=list(range(NCORES)),
                               trace=bool(int(os.environ.get("KD_TRACE", "0"))))
    _CACHE["last_result"] = res
    out = np.concatenate([r["out"] for r in res.results], axis=0)
    return out.astype(np.float32)


# revision 19
# speedup vs baseline: 1.0811x; 1.0811x over previous
"""Trainium2 Bass kernel: KDeep-style 3D CNN with atom->voxel splat.

Strategy (8 NeuronCores, pure data parallel, 2 batches/core):
  splat:  s = d^2 (via one K=5 matmul vs the voxel grid),
          x = (vr/d)^12 = Exp(-6*Ln(s) + 12*ln(vr)),  e = Exp(-x),  c = 1-e,
          lat[f,v] = sum_n h[n,f] * c[n,v]  (matmul over atoms)
  cnn:    conv via kernel-offset accumulation on the PE; fire concat([e,e])
          folded into halved squeeze weights; pools on DVE; final dot on PE.
"""
import os
import sys

import numpy as np

for _p in ("/opt/trn_rl_repo",):
    if _p not in sys.path:
        sys.path.insert(0, _p)

import ml_dtypes  # noqa: E402

BF = ml_dtypes.bfloat16

B, N1, N2, F, L = 16, 300, 40, 54, 24
NA = N1 + N2          # 340 atoms per batch (pos1 ++ pos2)
V = L ** 3            # 13824 voxels
NCORES, BPC = 8, 2    # batches per core
HALF = 12.0           # LATTICE_DIM / 2
NSTACK = 3            # grid stacks at partition bases 0/32/64
VSTACK = V // NSTACK  # 4608
VCH = 1024            # voxel chunk for elementwise splat work
ACH = (128, 128, 84)  # atom chunks
AOFF = (0, 128, 256)

# fire name -> (cin_raw, squeeze, expand)
FIRE = {'2': (96, 16, 64), '3': (128, 16, 64), '4': (128, 32, 128),
        '5': (256, 32, 128), '6': (256, 48, 192), '7': (384, 48, 192),
        '8': (384, 64, 256)}
# effective (folded) squeeze input channels
CIN_EFF = {'2': 96, '3': 64, '4': 64, '5': 128, '6': 128, '7': 192, '8': 192}

DEBUG_LAT = bool(int(os.environ.get("KD_DEBUG_LAT", "0")))

_CACHE = {}


def _f32(a):
    return np.ascontiguousarray(np.asarray(a, dtype=np.float32))


def _bf(a):
    return np.ascontiguousarray(np.asarray(a, dtype=np.float32).astype(BF))


def _host_consts(params):
    """Input-independent constant prep: grid table, folded/transposed weights."""
    c = {}
    # one-hot distance-gather table: row a*24+t is 1 where coordinate a of
    # voxel v equals t. d2[n,v] = sum_rows D2[n,row] * onehot[row,v] is exact.
    idx = np.arange(L, dtype=np.float32)
    gx, gy, gz = np.meshgrid(idx, idx, idx, indexing='ij')
    g = np.stack([gx.ravel(), gy.ravel(), gz.ravel()], 0)         # [3, V]
    oh = np.zeros((72, V), np.float32)
    for a in range(3):
        for t in range(L):
            oh[a * L + t] = (g[a] == t)
    c["onehot"] = _bf(oh)
    c["iota72"] = _f32((np.arange(72) % L).reshape(72, 1))

    def lhsT(w):  # [O, I, 3,3,3] -> [I, 27, O]
        w = np.asarray(w, np.float32)
        o, i = w.shape[:2]
        return np.transpose(w.reshape(o, i, 27), (1, 2, 0))

    w1 = np.asarray(params['conv1_w'], np.float32)                # [96,54,2,2,2]
    c["w_c1"] = _bf(np.transpose(w1.reshape(96, 54, 8), (1, 2, 0)))  # [54,8,96]
    c["b_c1"] = _f32(params['conv1_b']).reshape(-1, 1)

    for n, (cin_raw, sq, ex) in FIRE.items():
        wsq = np.asarray(params[f'sq{n}_w'], np.float32)
        if n != '2':
            e = CIN_EFF[n]
            wsq = wsq[:, :e] + wsq[:, e:]
        wt = lhsT(wsq)                                            # [cin_eff,27,sq]
        if wt.shape[0] > 128:
            c[f"w_sq{n}_0"] = _bf(wt[:128])
            c[f"w_sq{n}_1"] = _bf(wt[128:])
        else:
            c[f"w_sq{n}_0"] = _bf(wt)
        c[f"b_sq{n}"] = _f32(params[f'sq{n}_b']).reshape(-1, 1)
        c[f"w_ex{n}_0"] = _bf(lhsT(np.asarray(params[f'ex{n}_w'], np.float32)))
        be = _f32(params[f'ex{n}_b']).reshape(-1, 1)
        for i in range(0, be.shape[0], 128):
            c[f"b_ex{n}_{i // 128}"] = be[i:i + 128]

    lw = np.asarray(params['lin_w'], np.float32).reshape(512, 8)
    lw2 = (lw[:256] + lw[256:]) / 27.0                            # [256, 8]
    c["lw_0"] = _f32(lw2[:128])
    c["lw_1"] = _f32(lw2[128:])
    c["lin_b"] = _f32(params['lin_b']).reshape(1, 1)
    c["ones128"] = _f32(np.ones((128, 1)))
    return c


def _build_nc():
    import concourse.bacc as bacc
    import concourse.tile as tile
    from concourse import mybir

    DT = mybir.dt
    AF = mybir.ActivationFunctionType
    ALU = mybir.AluOpType

    nc = bacc.Bacc("TRN2", target_bir_lowering=False, debug=False,
                   num_devices=NCORES)

    # ------------------- DRAM I/O -------------------
    p72_d = nc.dram_tensor("p72", [BPC, 72, NA], DT.float32, kind="ExternalInput")
    vr_d = nc.dram_tensor("vr", [BPC, NA, 1], DT.float32, kind="ExternalInput")
    hbf_d = nc.dram_tensor("hbf", [BPC, NA, F], DT.bfloat16, kind="ExternalInput")
    onehot_d = nc.dram_tensor("onehot", [72, V], DT.bfloat16, kind="ExternalInput")
    iota72_d = nc.dram_tensor("iota72", [72, 1], DT.float32, kind="ExternalInput")
    w_c1_d = nc.dram_tensor("w_c1", [54, 8, 96], DT.bfloat16, kind="ExternalInput")
    b_c1_d = nc.dram_tensor("b_c1", [96, 1], DT.float32, kind="ExternalInput")
    wd, bd = {}, {}
    for n, (cin_raw, sq, ex) in FIRE.items():
        ce = CIN_EFF[n]
        if ce > 128:
            wd[f"sq{n}"] = [nc.dram_tensor(f"w_sq{n}_0", [128, 27, sq], DT.bfloat16, kind="ExternalInput"),
                            nc.dram_tensor(f"w_sq{n}_1", [ce - 128, 27, sq], DT.bfloat16, kind="ExternalInput")]
        else:
            wd[f"sq{n}"] = [nc.dram_tensor(f"w_sq{n}_0", [ce, 27, sq], DT.bfloat16, kind="ExternalInput")]
        wd[f"ex{n}"] = [nc.dram_tensor(f"w_ex{n}_0", [sq, 27, ex], DT.bfloat16, kind="ExternalInput")]
        bd[f"sq{n}"] = [nc.dram_tensor(f"b_sq{n}", [sq, 1], DT.float32, kind="ExternalInput")]
        bd[f"ex{n}"] = [nc.dram_tensor(f"b_ex{n}_{i // 128}",
                                       [min(128, ex - i), 1], DT.float32,
                                       kind="ExternalInput")
                        for i in range(0, ex, 128)]
    lw0_d = nc.dram_tensor("lw_0", [128, 8], DT.float32, kind="ExternalInput")
    lw1_d = nc.dram_tensor("lw_1", [128, 8], DT.float32, kind="ExternalInput")
    linb_d = nc.dram_tensor("lin_b", [1, 1], DT.float32, kind="ExternalInput")
    ones128_d = nc.dram_tensor("ones128", [128, 1], DT.float32, kind="ExternalInput")
    out_d = nc.dram_tensor("out", [BPC, 1], DT.float32, kind="ExternalOutput")
    if DEBUG_LAT:
        latdbg_d = nc.dram_tensor("lat_dbg", [F, BPC * V], DT.bfloat16,
                                  kind="ExternalOutput")

    with tile.TileContext(nc) as tc:
        _li = mybir.InstLoadActFuncSet(name=nc.get_next_instruction_name(),
                                       act_func_set_id=6, ins=[], outs=[])
        nc.scalar.add_instruction(_li)
        with (
            tc.tile_pool(name="const", bufs=1) as cpool,
            tc.tile_pool(name="prep", bufs=1) as prep,
            tc.tile_pool(name="ew", bufs=2) as ew,
            tc.tile_pool(name="cbf", bufs=2) as cbf,
            tc.tile_pool(name="lat", bufs=1) as latp,
            tc.tile_pool(name="cnn", bufs=3) as cnnp,
            tc.tile_pool(name="e4p", bufs=1) as e4p,
            tc.tile_pool(name="mpp", bufs=2) as mpp,
            tc.tile_pool(name="p7p", bufs=6) as p7p,
            tc.tile_pool(name="e8p", bufs=2) as e8p,
            tc.tile_pool(name="app", bufs=2) as app,
            tc.tile_pool(name="small", bufs=4) as smallp,
            tc.tile_pool(name="ps_s", bufs=2, space="PSUM") as ps_s,
            tc.tile_pool(name="ps_lat", bufs=1, space="PSUM") as ps_lat,
            tc.tile_pool(name="ps_cnn", bufs=2, space="PSUM") as ps_cnn,
        ):
            # ---------------- constants in ----------------
            oh_t = cpool.tile([72, V], DT.bfloat16, tag="onehot")
            nc.sync.dma_start(oh_t[:], onehot_d[:])
            iota72_t = cpool.tile([72, 1], DT.float32, tag="iota72")
            nc.sync.dma_start(iota72_t[:], iota72_d[:])
            wc1_t = cpool.tile([54, 8, 96], DT.bfloat16, tag="wc1")
            nc.sync.dma_start(wc1_t[:], w_c1_d[:])
            bc1_t = cpool.tile([96, 1], DT.float32, tag="bc1")
            nc.sync.dma_start(bc1_t[:], b_c1_d[:])
            wt, bt = {}, {}
            for key, dlist in wd.items():
                wt[key] = []
                for i, d in enumerate(dlist):
                    t = cpool.tile(list(d.shape), DT.bfloat16, tag=f"w{key}{i}")
                    nc.sync.dma_start(t[:], d[:])
                    wt[key].append(t)
            for key, dlist in bd.items():
                bt[key] = []
                for i, d in enumerate(dlist):
                    t = cpool.tile(list(d.shape), DT.float32, tag=f"b{key}{i}")
                    nc.sync.dma_start(t[:], d[:])
                    bt[key].append(t)
            lw_t0 = cpool.tile([128, 8], DT.float32, tag="lw0")
            lw_t1 = cpool.tile([128, 8], DT.float32, tag="lw1")
            lw_t = [lw_t0, lw_t1]
            nc.sync.dma_start(lw_t[0][:], lw0_d[:])
            nc.sync.dma_start(lw_t[1][:], lw1_d[:])
            linb_t = cpool.tile([1, 1], DT.float32, tag="linb")
            nc.sync.dma_start(linb_t[:], linb_d[:])
            ones128_t = cpool.tile([128, 1], DT.float32, tag="ones128")
            nc.sync.dma_start(ones128_t[:], ones128_d[:])

            # ---------------- per-batch atom prep ----------------
            # D2 rows a*24+t = (p_a + shift_a - t)^2, split hi/mid/lo into
            # three bf16 tables so the one-hot matmul reconstructs exact d^2.
            qhi_t = prep.tile([72, BPC * NA], DT.bfloat16, tag="qhi")
            qr1_t = prep.tile([72, BPC * NA], DT.bfloat16, tag="qr1")
            qr2_t = prep.tile([72, BPC * NA], DT.bfloat16, tag="qr2")
            bias12 = {}   # (b, ac) -> [aclen,1] tile with 12*ln(vr)
            h_t = {}      # (b, ac) -> [aclen, F] bf16
            for b in range(BPC):
                p72 = prep.tile([72, NA], DT.float32, tag="p72")
                nc.sync.dma_start(p72[:], p72_d[b])
                bmax = smallp.tile([72, 1], DT.float32, tag="bmax")
                bmin = smallp.tile([72, 1], DT.float32, tag="bmin")
                nc.vector.tensor_reduce(bmax[:], p72[:, :N1], axis=mybir.AxisListType.X,
                                        op=ALU.max)
                nc.vector.tensor_reduce(bmin[:], p72[:, :N1], axis=mybir.AxisListType.X,
                                        op=ALU.min)
                sh = smallp.tile([72, 1], DT.float32, tag="sh")
                nc.vector.tensor_add(sh[:], bmax[:], bmin[:])
                # shift = -(bmin+bmax)/2 + HALF; bias row = shift_a - t
                nc.vector.tensor_scalar(sh[:], sh[:], -0.5, HALF, ALU.mult, ALU.add)
                nc.vector.tensor_sub(sh[:], sh[:], iota72_t[:])
                d2f = prep.tile([72, NA], DT.float32, tag="d2f")
                nc.scalar.activation(d2f[:], p72[:], AF.Square, bias=sh[:])
                bsl = slice(b * NA, (b + 1) * NA)
                nc.vector.tensor_copy(qhi_t[:, bsl], d2f[:])
                r1 = prep.tile([72, NA], DT.float32, tag="r1")
                nc.vector.tensor_sub(r1[:], d2f[:], qhi_t[:, bsl])
                nc.vector.tensor_copy(qr1_t[:, bsl], r1[:])
                nc.vector.tensor_sub(r1[:], r1[:], qr1_t[:, bsl])
                nc.vector.tensor_copy(qr2_t[:, bsl], r1[:])

                for ac in range(3):
                    al, ao = ACH[ac], AOFF[ac]
                    vrt = smallp.tile([al, 1], DT.float32, tag="vrt")
                    nc.sync.dma_start(vrt[:], vr_d[b, ao:ao + al])
                    vr2 = smallp.tile([al, 1], DT.float32, tag="vr2")
                    nc.vector.tensor_mul(vr2[:], vrt[:], vrt[:])
                    lnv = smallp.tile([al, 1], DT.float32, tag="lnv")
                    nc.scalar.activation(lnv[:], vr2[:], AF.Ln)
                    b12 = prep.tile([al, 1], DT.float32, tag=f"b12_{b}_{ac}")
                    nc.vector.tensor_scalar_mul(b12[:], lnv[:], 6.0)
                    bias12[(b, ac)] = b12
                    ht = prep.tile([al, F], DT.bfloat16, tag=f"h_{b}_{ac}")
                    nc.sync.dma_start(ht[:], hbf_d[b, ao:ao + al, :])
                    h_t[(b, ac)] = ht

            # ---------------- splat ----------------
            lat_t0 = latp.tile([F, L, L, L], DT.bfloat16, tag="lat0")
            lat_t1 = latp.tile([F, L, L, L], DT.bfloat16, tag="lat1")
            lat_bt = [lat_t0, lat_t1]
            lat_fls = [t[:].rearrange("f i j k -> f (i j k)") for t in lat_bt]
            for b in range(BPC):
                voff = 0
                while voff < V:
                    cl = min(VCH, V - voff)
                    latps = ps_lat.tile([F, VCH], DT.float32, tag="latps")
                    for ac in range(3):
                        al, ao = ACH[ac], AOFF[ac]
                        asl = slice(b * NA + ao, b * NA + ao + al)
                        sps = ps_s.tile([128, VCH], DT.float32, tag="sps")
                        for j in range(0, cl, 512):
                            je = min(j + 512, cl)
                            for si, qt in enumerate((qhi_t, qr1_t, qr2_t)):
                                nc.tensor.matmul(
                                    sps[:al, j:je], qt[:, asl],
                                    oh_t[:, voff + j:voff + je],
                                    start=(si == 0), stop=(si == 2))
                        y = ew.tile([128, VCH], DT.float32, tag="ew")
                        nc.scalar.activation(y[:al, :cl], sps[:al, :cl], AF.Ln)
                        nc.scalar.activation(y[:al, :cl], y[:al, :cl], AF.Exp,
                                             bias=bias12[(b, ac)][:],
                                             scale=-6.0)
                        nc.scalar.activation(y[:al, :cl], y[:al, :cl], AF.Exp,
                                             scale=-1.0)
                        ct = cbf.tile([128, VCH], DT.bfloat16, tag="c")
                        nc.vector.tensor_scalar(ct[:al, :cl], y[:al, :cl],
                                                -1.0, 1.0, ALU.mult, ALU.add)
                        for j in range(0, cl, 512):
                            je = min(j + 512, cl)
                            nc.tensor.matmul(
                                latps[:, j:je], h_t[(b, ac)][:],
                                ct[:al, j:je],
                                start=(ac == 0), stop=(ac == 2))
                    nc.vector.tensor_copy(lat_fls[b][:, voff:voff + cl],
                                          latps[:, :cl])
                    voff += cl
            if DEBUG_LAT:
                nc.sync.dma_start(latdbg_d[:, :V], lat_fls[0][:])
                nc.sync.dma_start(latdbg_d[:, V:], lat_fls[1][:])

            # ---------------- CNN ----------------
            def conv_to(psum_pool, in_views, w_tiles, mslice, n_views,
                        first_extra=False):
                """Accumulate sum over kchunks x 27 offsets into psum tiles.
                in_views: per kchunk, fn(t0,t1,t2, nv) -> rhs AP
                n_views:  list of (psum_tile, nv_key) output chunks."""
                for (pt, nv) in n_views:
                    first = True
                    for kc, wtile in enumerate(w_tiles):
                        for t0 in range(3):
                            for t1 in range(3):
                                for t2 in range(3):
                                    t27 = (t0 * 3 + t1) * 3 + t2
                                    rhs = in_views[kc](t0, t1, t2, nv)
                                    last = (kc == len(w_tiles) - 1 and t27 == 26)
                                    nc.tensor.matmul(
                                        pt[:], wtile[:, t27, mslice], rhs,
                                        start=first, stop=last)
                                    first = False

            for b in range(BPC):
                # conv1: lat [54, (i,j,k)] -> pad14 buffer x1 [96, 14^3]
                x1 = cnnp.tile([128, 14, 14, 14], DT.bfloat16, tag="pad14")
                nc.vector.memset(x1[:96], 0.0)
                for gchunk in range(4):
                    a0 = 3 * gchunk
                    pt = ps_cnn.tile([96, 3, 12, 12], DT.float32, tag="cps")
                    first = True
                    for tj in range(2):
                        for tk in range(2):
                            for ti in range(2):
                                t8 = (tj * 2 + tk) * 2 + ti
                                # out (dj,hk,wi): in i=2wi+ti, j=2dj+tj, k=2hk+tk
                                rhs = lat_bt[b][:, ti:24:2,
                                            tj + 2 * a0:tj + 2 * a0 + 5:2,
                                            tk:24:2]
                                rhs = rhs.rearrange("f i j k -> f j k i")
                                nc.tensor.matmul(pt[:], wc1_t[:, t8, :], rhs,
                                                 start=first, stop=(t8 == 7))
                                first = False
                    nc.scalar.activation(
                        x1[:96, 1 + a0:4 + a0, 1:13, 1:13], pt[:],
                        AF.Relu, bias=bc1_t[:])

                def pad14_views(t, cin):
                    def f(t0, t1, t2, nv):
                        a0 = nv
                        return t[:cin, a0 + t0:a0 + t0 + 3, t1:t1 + 12,
                                 t2:t2 + 12]
                    return f

                def fire12(name, in_t, cin):
                    """12^3 fire layer pair; returns expand output tile (pad14)."""
                    sq_c = FIRE[name][1]
                    ex_c = FIRE[name][2]
                    s_t = cnnp.tile([128, 14, 14, 14], DT.bfloat16, tag="pad14")
                    nc.vector.memset(s_t[:sq_c], 0.0)
                    for gch in range(4):
                        pt = ps_cnn.tile([sq_c, 3, 12, 12], DT.float32, tag="cps")
                        conv_to(ps_cnn, [pad14_views(in_t, cin)], wt[f"sq{name}"],
                                slice(0, sq_c), [(pt, 3 * gch)])
                        nc.scalar.activation(
                            s_t[:sq_c, 1 + 3 * gch:4 + 3 * gch, 1:13, 1:13],
                            pt[:], AF.Relu, bias=bt[f"sq{name}"][0][:])
                    if name == '4':
                        e_t = e4p.tile([128, 12, 12, 12], DT.bfloat16, tag="e4")
                    else:
                        e_t = cnnp.tile([128, 14, 14, 14], DT.bfloat16, tag="pad14")
                        nc.vector.memset(e_t[:ex_c], 0.0)
                    for gch in range(4):
                        pt = ps_cnn.tile([ex_c, 3, 12, 12], DT.float32, tag="cps")
                        conv_to(ps_cnn, [pad14_views(s_t, sq_c)], wt[f"ex{name}"],
                                slice(0, ex_c), [(pt, 3 * gch)])
                        if name == '4':
                            nc.scalar.activation(
                                e_t[:ex_c, 3 * gch:3 * gch + 3, :, :], pt[:],
                                AF.Relu, bias=bt[f"ex{name}"][0][:])
                        else:
                            nc.scalar.activation(
                                e_t[:ex_c, 1 + 3 * gch:4 + 3 * gch, 1:13, 1:13],
                                pt[:], AF.Relu, bias=bt[f"ex{name}"][0][:])
                    return e_t

                e2 = fire12('2', x1, 96)
                e3 = fire12('3', e2, 64)
                e4 = fire12('4', e3, 64)

                # maxpool(2,3,1): 12 -> 5 per axis, into pad7 [128,7,7,7]
                t1 = mpp.tile([128, 12, 12, 5], DT.bfloat16, tag="mp1")
                nc.vector.tensor_max(t1[:, :, :, 1:4], e4[:, :, :, 2:11:3],
                                     e4[:, :, :, 3:12:3])
                nc.vector.tensor_copy(t1[:, :, :, 0:1], e4[:, :, :, 0:1])
                nc.vector.tensor_copy(t1[:, :, :, 4:5], e4[:, :, :, 11:12])
                t2 = mpp.tile([128, 12, 5, 5], DT.bfloat16, tag="mp2")
                nc.vector.tensor_max(t2[:, :, 1:4, :], t1[:, :, 2:11:3, :],
                                     t1[:, :, 3:12:3, :])
                nc.vector.tensor_copy(t2[:, :, 0:1, :], t1[:, :, 0:1, :])
                nc.vector.tensor_copy(t2[:, :, 4:5, :], t1[:, :, 11:12, :])
                m4 = p7p.tile([128, 7, 7, 7], DT.bfloat16, tag="pad7")
                nc.vector.memset(m4[:], 0.0)
                nc.vector.tensor_max(m4[:, 2:5, 1:6, 1:6], t2[:, 2:11:3, :, :],
                                     t2[:, 3:12:3, :, :])
                nc.vector.tensor_copy(m4[:, 1:2, 1:6, 1:6], t2[:, 0:1, :, :])
                nc.vector.tensor_copy(m4[:, 5:6, 1:6, 1:6], t2[:, 11:12, :, :])

                def pad7_views(t, cin):
                    def f(t0, t1, t2, nv):
                        return t[:cin, t0:t0 + 5, t1:t1 + 5, t2:t2 + 5]
                    return f

                def fire5(name, in_tiles):
                    """5^3 fire; in_tiles: list of (pad7 tile, cin) kchunks.
                    Returns list of output (tile, ch) (plain tiles for f8)."""
                    sq_c = FIRE[name][1]
                    ex_c = FIRE[name][2]
                    s_t = p7p.tile([128, 7, 7, 7], DT.bfloat16, tag="pad7")
                    nc.vector.memset(s_t[:sq_c], 0.0)
                    pt = ps_cnn.tile([sq_c, 5, 5, 5], DT.float32, tag="cps")
                    conv_to(ps_cnn, [pad7_views(t, cc) for (t, cc) in in_tiles],
                            wt[f"sq{name}"], slice(0, sq_c), [(pt, 0)])
                    nc.scalar.activation(s_t[:sq_c, 1:6, 1:6, 1:6], pt[:],
                                         AF.Relu, bias=bt[f"sq{name}"][0][:])
                    outs = []
                    m0 = 0
                    while m0 < ex_c:
                        mc = min(128, ex_c - m0)
                        if name == '8':
                            o_t = e8p.tile([128, 5, 5, 5], DT.bfloat16, tag="e8")
                            ov = o_t[:mc, :, :, :]
                        else:
                            o_t = p7p.tile([128, 7, 7, 7], DT.bfloat16, tag="pad7")
                            nc.vector.memset(o_t[:mc], 0.0)
                            ov = o_t[:mc, 1:6, 1:6, 1:6]
                        pt = ps_cnn.tile([mc, 5, 5, 5], DT.float32, tag="cps")
                        conv_to(ps_cnn, [pad7_views(s_t, sq_c)], wt[f"ex{name}"],
                                slice(m0, m0 + mc), [(pt, 0)])
                        nc.scalar.activation(ov, pt[:], AF.Relu,
                                             bias=bt[f"ex{name}"][m0 // 128][:])
                        outs.append((o_t, mc))
                        m0 += mc
                    return outs

                f5 = fire5('5', [(m4, 128)])
                f6 = fire5('6', f5)
                f7 = fire5('7', f6)
                f8 = fire5('8', f7)

                # avgpool(3,2,0): 5 -> 2 per axis (sum; 1/27 folded into lw)
                fin = ps_cnn.tile([1, 1], DT.float32, tag="cps")
                for hh, (e8, _mc) in enumerate(f8):
                    u1 = app.tile([128, 5, 5, 2], DT.float32, tag="ap1")
                    nc.vector.tensor_add(u1[:], e8[:, :, :, 0:4:2], e8[:, :, :, 1:5:2])
                    nc.vector.tensor_add(u1[:], u1[:], e8[:, :, :, 2:5:2])
                    u2 = app.tile([128, 5, 2, 2], DT.float32, tag="ap2")
                    nc.vector.tensor_add(u2[:], u1[:, :, 0:4:2, :], u1[:, :, 1:5:2, :])
                    nc.vector.tensor_add(u2[:], u2[:], u1[:, :, 2:5:2, :])
                    u3 = app.tile([128, 2, 2, 2], DT.float32, tag="ap3")
                    nc.vector.tensor_add(u3[:], u2[:, 0:4:2, :, :], u2[:, 1:5:2, :, :])
                    nc.vector.tensor_add(u3[:], u3[:], u2[:, 2:5:2, :, :])
                    # dot with folded linear weights
                    m1 = app.tile([128, 8], DT.float32, tag="lm")
                    nc.vector.tensor_mul(m1[:], u3[:].rearrange("p a b c -> p (a b c)"),
                                         lw_t[hh][:])
                    red = app.tile([128, 1], DT.float32, tag="lred")
                    nc.vector.tensor_reduce(red[:], m1[:], axis=mybir.AxisListType.X,
                                            op=ALU.add)
                    nc.tensor.matmul(fin[:], red[:], ones128_t[:],
                                     start=(hh == 0), stop=(hh == 1))
                ob = smallp.tile([1, 1], DT.float32, tag="ob")
                nc.scalar.activation(ob[:], fin[:], AF.Identity, bias=linb_t[:])
                nc.sync.dma_start(out_d[b:b + 1, :], ob[:])

    nc.compile()
    return nc


def _get_nc():
    if "nc" not in _CACHE:
        _CACHE["nc"] = _build_nc()
    return _CACHE["nc"]


def kernel(pos1, pos2, vr1, vr2, h1, h2, params):
    from concourse.bass_utils import run_bass_kernel_spmd

    pos1, pos2 = _f32(pos1), _f32(pos2)
    vr1, vr2 = _f32(vr1), _f32(vr2)
    h1, h2 = _f32(h1), _f32(h2)

    consts = _host_consts(params)
    nc = _get_nc()

    posT = np.concatenate([pos1, pos2], axis=1).transpose(0, 2, 1)  # [B,3,NA]
    p72 = np.repeat(posT, L, axis=1)                                # [B,72,NA]
    vr = np.concatenate([vr1, vr2], axis=1)[..., None]              # [B,NA,1]
    hbf = np.concatenate([h1, h2], axis=1).astype(BF)               # [B,NA,F]

    in_maps = []
    for c in range(NCORES):
        b0 = c * BPC
        m = dict(consts)
        m["p72"] = _f32(p72[b0:b0 + BPC])
        m["vr"] = _f32(vr[b0:b0 + BPC])
        m["hbf"] = np.ascontiguousarray(hbf[b0:b0 + BPC])
        in_maps.append(m)

    res = run_bass_kernel_spmd(nc, in_maps, core_ids=list(range(NCORES)),
                               trace=bool(int(os.environ.get("KD_TRACE", "0"))))
    _CACHE["last_result"] = res
    out = np.concatenate([r["out"] for r in res.results], axis=0)
    return out.astype(np.float32)
